# revision 9
# baseline (speedup 1.0000x reference)
"""HQQ-quantized linear + LoRA (nn_HQQLinearLoRA) on 8 trn2 NeuronCores.

  out = x @ ((W_q - zero)*scale)^T + (x @ lora_A @ lora_B) * 2.0 + bias

Sharding: 4 token-groups (batch dim) x 2 out-feature-groups = 8 cores.
Each core computes out[b, :, og*2048:(og+1)*2048] for its (b, og).

Host passes pre-transposed f16 views (layout + dtype staging only):
  - xT16  [4096, 2048] f16 : x[b].T, rows permuted so each 128-row k-tile's
          partition p maps to quant-group g = p % 64 (uniform across tiles)
  - wqT16 [4096, 2048] f16 : W_q[o-shard].T with the same row permutation
          (W_q values 0..15 are exact in f16)
  - scaleT/zeroT [64, 2048] f16, la16 [128, KT*R] f16 (permuted, pre-scaled
    by 2.0), lb16 [R+1, 2048] f16 (lora_B rows + bias row), ones [1, 2048]

All DMAs are plain HWDGE transfers (nc.sync / nc.scalar) — no casts, no
SWDGE. Device: dequant W on DVE using a [128, 2048] scale/zero tile (valid
for every k-tile thanks to the permutation); f16 matmul with f32 PSUM
accumulation; LoRA + bias fold into the same PSUM accumulation as one K=17
matmul.
"""

import sys

import numpy as np

sys.path.append("/opt/trn_rl_repo")

import concourse.bass as bass  # noqa: E402
import concourse.mybir as mybir  # noqa: E402
import concourse.tile as tile  # noqa: E402
from concourse import bacc  # noqa: E402
from concourse.bass_utils import run_bass_kernel_spmd  # noqa: E402

B, S, I, O, R = 4, 2048, 4096, 4096, 16
GS = 64
G = I // GS  # 64
NCORES = 8
OG = 2
O_SH = O // OG  # 2048
T = S  # 2048 tokens per core
KT = I // 128  # 32 k-tiles
TCH = 512  # token chunk
NTCH = T // TCH  # 4
OCH = 512  # o quarter (dequant-W granule)
NOCH = O_SH // OCH  # 4
SCALING = 2.0

F32 = mybir.dt.float32
F16 = mybir.dt.float16
I32 = mybir.dt.int32

TRACE = False
TRACE_KWARGS = {}
LAST_RESULTS = None


def _perm() -> np.ndarray:
    """Row order such that k-tile k, partition p holds input-feature
    i = (p % 64)*64 + 2k + p//64, i.e. quant group g(i) = p % 64."""
    p = np.arange(128)
    out = np.empty(I, dtype=np.int64)
    for k in range(KT):
        out[k * 128 + p] = (p % 64) * 64 + 2 * k + p // 64
    return out


PERM = _perm()

_nc_cache = None


def _build():
    nc = bacc.Bacc(None)
    # xS[tci, p, k*TCH + t] = x[b, tci*TCH + t, PERM[k*128+p]]  (f16)
    xS_d = nc.dram_tensor("xS", [NTCH, 128, KT * TCH], F16, kind="ExternalInput")
    # wqS[oq, p, k*OCH + o] = W_q[og*O_SH + oq*OCH + o, PERM[k*128+p]]  (f16)
    wqS_d = nc.dram_tensor("wqS", [NOCH, 128, KT * OCH], F16, kind="ExternalInput")
    scaleT_d = nc.dram_tensor("scaleT", [G, O_SH], F16, kind="ExternalInput")
    zeroT_d = nc.dram_tensor("zeroT", [G, O_SH], F16, kind="ExternalInput")
    la_d = nc.dram_tensor("la", [128, KT * R], F16, kind="ExternalInput")
    lb_d = nc.dram_tensor("lb", [R + 1, O_SH], F16, kind="ExternalInput")
    ones_d = nc.dram_tensor("ones", [1, T], F16, kind="ExternalInput")
    out_d = nc.dram_tensor("out", [NOCH, T, OCH], F32, kind="ExternalOutput")

    with tile.TileContext(nc) as tc:
        with (
            tc.tile_pool(name="const", bufs=1) as constp,
            tc.tile_pool(name="w16", bufs=3) as w16p,
            tc.tile_pool(name="d16", bufs=2) as d16p,
            tc.tile_pool(name="x16", bufs=2) as x16p,
            tc.tile_pool(name="ob", bufs=3) as obp,
            tc.tile_pool(name="ps", bufs=4, space="PSUM") as psp,
            tc.tile_pool(name="psl", bufs=2, space="PSUM") as pslp,
        ):
            # ---- constants ----
            # scale/zero expanded: row p <- scaleT[p % 64, :]
            s16 = constp.tile([128, O_SH], F16)
            z16 = constp.tile([128, O_SH], F16)
            for h in (0, 1):
                nc.sync.dma_start(s16[64 * h : 64 * h + 64, :], scaleT_d[:, :])
                nc.sync.dma_start(z16[64 * h : 64 * h + 64, :], zeroT_d[:, :])
            # lora_A tiles (pre-scaled by 2.0 on host): [128, (k r)]
            la16 = constp.tile([128, KT, R], F16)
            nc.sync.dma_start(la16[:], la_d.rearrange("p (k r) -> p k r", k=KT))
            # [loraB; bias] rhs for the fused K=17 matmul
            lb16 = constp.tile([R + 1, O_SH], F16)
            nc.sync.dma_start(lb16[:], lb_d[:])
            # [t1; ones] lhsT rows; row R stays 1.0
            t1sb = constp.tile([R + 1, T], F16)
            nc.sync.dma_start(t1sb[R : R + 1, :], ones_d[:])

            for oh in range(2):  # o-halves (x streamed once per half)
                w16q_list = []
                for qq in range(2):  # dequant W in o-quarters of 512
                    oq = oh * 2 + qq
                    w16q = w16p.tile([128, KT, OCH], F16)
                    nc.scalar.dma_start(
                        w16q[:],
                        wqS_d[oq].rearrange("p (k o) -> p k o", k=KT),
                    )
                    for k in range(KT):
                        d16 = d16p.tile([128, OCH], F16)
                        nc.vector.tensor_sub(
                            d16[:], w16q[:, k, :], z16[:, oq * OCH : (oq + 1) * OCH]
                        )
                        nc.vector.tensor_mul(
                            w16q[:, k, :],
                            d16[:],
                            s16[:, oq * OCH : (oq + 1) * OCH],
                        )
                    w16q_list.append(w16q)

                for tci in range(NTCH):
                    # one 4 MiB f16 DMA per token chunk, fully contiguous
                    x16 = x16p.tile([128, KT, TCH], F16)
                    nc.sync.dma_start(
                        x16[:],
                        xS_d[tci].rearrange("p (k t) -> p k t", k=KT),
                    )
                    if oh == 0:
                        t1ps = pslp.tile([R, TCH], F32)
                        for k in range(KT):
                            nc.tensor.matmul(
                                t1ps[:],
                                la16[:, k, :],
                                x16[:, k, :],
                                start=(k == 0),
                                stop=(k == KT - 1),
                            )
                        nc.vector.tensor_copy(
                            t1sb[0:R, tci * TCH : (tci + 1) * TCH], t1ps[:]
                        )
                    for tt in range(TCH // 128):
                        t0 = tci * TCH + tt * 128
                        for qq in range(2):
                            o0 = (oh * 2 + qq) * OCH
                            ps = psp.tile([128, OCH], F32)
                            for k in range(KT):
                                nc.tensor.matmul(
                                    ps[:],
                                    x16[:, k, tt * 128 : tt * 128 + 128],
                                    w16q_list[qq][:, k, :],
                                    start=(k == 0),
                                    stop=False,
                                )
                            nc.tensor.matmul(
                                ps[:],
                                t1sb[:, t0 : t0 + 128],
                                lb16[:, o0 : o0 + OCH],
                                start=False,
                                stop=True,
                            )
                            ob = obp.tile([128, OCH], F32)
                            nc.vector.tensor_copy(ob[:], ps[:])
                            nc.scalar.dma_start(
                                out_d[oh * 2 + qq, t0 : t0 + 128, :], ob[:]
                            )

    nc.compile()
    return nc


def make_in_maps(x, W_q, scale, zero, lora_A, lora_B, bias):
    x = np.asarray(x, dtype=np.float32)
    W_q = np.asarray(W_q, dtype=np.int32)
    scale = np.asarray(scale, dtype=np.float32)
    zero = np.asarray(zero, dtype=np.float32)
    lora_A = np.asarray(lora_A, dtype=np.float32)
    lora_B = np.asarray(lora_B, dtype=np.float32)
    bias = np.asarray(bias, dtype=np.float32)

    # [128, KT*R] f16: la[p, k*R+r] = 2*lora_A[PERM[k*128+p], r]
    laP = (SCALING * lora_A[PERM]).astype(np.float16)  # [(k p), R]
    la16 = np.ascontiguousarray(
        laP.reshape(KT, 128, R).transpose(1, 0, 2).reshape(128, KT * R)
    )
    # xS per batch element (shared by the 2 o-group cores):
    # xS[tci, p, k*TCH+t] = x[b, tci*TCH+t, PERM[k*128+p]]
    xS_b = []
    for b in range(B):
        xT16 = x[b].T[PERM].astype(np.float16)  # [(k p), T]
        xS = (
            xT16.reshape(KT, 128, NTCH, TCH)
            .transpose(2, 1, 0, 3)
            .reshape(NTCH, 128, KT * TCH)
        )
        xS_b.append(np.ascontiguousarray(xS))
    ones16 = np.ones((1, T), dtype=np.float16)

    in_maps = []
    for c in range(NCORES):
        b, og = c // OG, c % OG
        osl = slice(og * O_SH, (og + 1) * O_SH)
        lb16 = np.empty((R + 1, O_SH), dtype=np.float16)
        lb16[0:R] = lora_B[:, osl]
        lb16[R] = bias[osl]
        # wqS[oq, p, k*OCH+o] = W_q[og*O_SH + oq*OCH + o, PERM[k*128+p]]
        wqT16 = W_q[osl].T[PERM].astype(np.float16)  # [(k p), O_SH]
        wqS = (
            wqT16.reshape(KT, 128, NOCH, OCH)
            .transpose(2, 1, 0, 3)
            .reshape(NOCH, 128, KT * OCH)
        )
        in_maps.append(
            {
                "xS": xS_b[b],
                "wqS": np.ascontiguousarray(wqS),
                "scaleT": np.ascontiguousarray(scale[osl].T.astype(np.float16)),
                "zeroT": np.ascontiguousarray(zero[osl].T.astype(np.float16)),
                "la": la16,
                "lb": lb16,
                "ones": ones16,
            }
        )
    return in_maps


def kernel(x, W_q, scale, zero, lora_A, lora_B, bias):
    global _nc_cache, LAST_RESULTS
    if _nc_cache is None:
        _nc_cache = _build()
    nc = _nc_cache

    in_maps = make_in_maps(x, W_q, scale, zero, lora_A, lora_B, bias)

    res = run_bass_kernel_spmd(
        nc,
        in_maps,
        core_ids=list(range(NCORES)),
        trace=TRACE,
        trace_kwargs=TRACE_KWARGS,
    )
    LAST_RESULTS = res

    out = np.empty((B, S, O), dtype=np.float32)
    for c in range(NCORES):
        b, og = c // OG, c % OG
        o_c = res.results[c]["out"]  # [NOCH, T, OCH]
        for q in range(NOCH):
            o0 = og * O_SH + q * OCH
            out[b, :, o0 : o0 + OCH] = o_c[q]
    return out


# revision 11
# speedup vs baseline: 1.0409x; 1.0409x over previous
"""HQQ-quantized linear + LoRA (nn_HQQLinearLoRA) on 8 trn2 NeuronCores.

  out = x @ ((W_q - zero)*scale)^T + (x @ lora_A @ lora_B) * 2.0 + bias

Sharding: 4 token-groups (batch dim) x 2 out-feature-groups = 8 cores.
Each core computes out[b, :, og*2048:(og+1)*2048] for its (b, og).

Zero-point fold: (W_q - z)*s = W_q*s - z*s, so
  out = x @ (W_q*s)^T - xg @ (z*s)^T + 2*(x @ A) @ B + bias
where xg[t,g] = sum_{i in group g} x[t,i].  Device dequant is then a single
DVE multiply, and xg rides along with the LoRA intermediate: one matmul per
k-tile with lhsT = [2A | group-indicator] produces [t1 | xg] in PSUM, and
one K=81 matmul per output tile applies [lora_B; -(z*s)^T; bias].

Host passes pre-packed f16 views (layout + dtype staging only; W_q values
0..15 are exact in f16).  Rows are permuted so k-tile partition p holds
quant-group g = p % 64 (uniform across k-tiles).  All DMAs are plain HWDGE
transfers (nc.sync / nc.scalar) with per-partition-contiguous descriptors —
no casts, no SWDGE.

All 4 dequantized W quarters stay resident in SBUF (x streamed once);  the
main-matmul stream is software-pipelined one token-chunk behind the
[t1|xg] stage so W loads + dequant hide under the pipeline fill.
"""

import sys

import numpy as np

sys.path.append("/opt/trn_rl_repo")

import concourse.bass as bass  # noqa: E402
import concourse.mybir as mybir  # noqa: E402
import concourse.tile as tile  # noqa: E402
from concourse import bacc  # noqa: E402
from concourse.bass_utils import run_bass_kernel_spmd  # noqa: E402

B, S, I, O, R = 4, 2048, 4096, 4096, 16
GS = 64
G = I // GS  # 64
NCORES = 8
OG = 2
O_SH = O // OG  # 2048
T = S  # 2048 tokens per core
KT = I // 128  # 32 k-tiles
TCH = 256  # token chunk
NTCH = T // TCH  # 8
OCH = 512  # o quarter (dequant-W granule)
NOCH = O_SH // OCH  # 4
SCALING = 2.0
CW = R + G  # 80: [t1 | xg] columns
CK = CW + 1  # 81: correction contraction (incl. ones row for bias)

F32 = mybir.dt.float32
F16 = mybir.dt.float16

TRACE = False
TRACE_KWARGS = {}
LAST_RESULTS = None


def _perm() -> np.ndarray:
    """Row order such that k-tile k, partition p holds input-feature
    i = (p % 64)*64 + 2k + p//64, i.e. quant group g(i) = p % 64."""
    p = np.arange(128)
    out = np.empty(I, dtype=np.int64)
    for k in range(KT):
        out[k * 128 + p] = (p % 64) * 64 + 2 * k + p // 64
    return out


PERM = _perm()

_nc_cache = None


def _build():
    nc = bacc.Bacc(None)
    # xS[tci, p, k*TCH + t] = x[b, tci*TCH + t, PERM[k*128+p]]  (f16)
    xS_d = nc.dram_tensor("xS", [NTCH, 128, KT * TCH], F16, kind="ExternalInput")
    # wqS[oq, p, k*OCH + o] = W_q[og*O_SH + oq*OCH + o, PERM[k*128+p]]  (f16)
    wqS_d = nc.dram_tensor("wqS", [NOCH, 128, KT * OCH], F16, kind="ExternalInput")
    scaleT_d = nc.dram_tensor("scaleT", [G, O_SH], F16, kind="ExternalInput")
    # laind[p, k*CW + 0:R] = 2*lora_A[PERM[k*128+p], :];  [.., R+g] = (p%64==g)
    laind_d = nc.dram_tensor("laind", [128, KT * CW], F16, kind="ExternalInput")
    # corr rows: 0..R-1 = lora_B, R+g = -(zero*scale)^T, R+G = bias
    corr_d = nc.dram_tensor("corr", [CK, O_SH], F16, kind="ExternalInput")
    ones_d = nc.dram_tensor("ones", [1, T], F16, kind="ExternalInput")
    out_d = nc.dram_tensor("out", [NOCH, T, OCH], F32, kind="ExternalOutput")

    Copy = mybir.ActivationFunctionType.Copy

    with tile.TileContext(nc) as tc:
        with (
            tc.tile_pool(name="const", bufs=1) as constp,
            tc.tile_pool(name="w16", bufs=4) as w16p,
            tc.tile_pool(name="x16", bufs=3) as x16p,
            tc.tile_pool(name="ob", bufs=4) as obp,
            tc.tile_pool(name="ps", bufs=6, space="PSUM") as psp,
            tc.tile_pool(name="psl", bufs=2, space="PSUM") as pslp,
        ):
            # ---- constants (SP ring; small) ----
            s16 = constp.tile([128, O_SH], F16)
            for h in (0, 1):
                nc.sync.dma_start(s16[64 * h : 64 * h + 64, :], scaleT_d[:, :])
            laind = constp.tile([128, KT, CW], F16)
            nc.sync.dma_start(laind[:], laind_d.rearrange("p (k c) -> p k c", k=KT))
            corr16 = constp.tile([CK, O_SH], F16)
            nc.sync.dma_start(corr16[:], corr_d[:])
            # [t1 | xg ; ones] lhsT rows; row CW stays 1.0
            t1sb = constp.tile([CK, T], F16)
            nc.sync.dma_start(t1sb[CW : CW + 1, :], ones_d[:])

            # ---- W quarters: DMA in 2 halves (ACT ring) + 1-op dequant ----
            w16q_list = []
            for oq in range(NOCH):
                w16q = w16p.tile([128, KT, OCH], F16)
                kh = KT // 2
                for h in (0, 1):
                    nc.scalar.dma_start(
                        w16q[:, h * kh : (h + 1) * kh, :],
                        wqS_d[oq, :, h * kh * OCH : (h + 1) * kh * OCH].rearrange(
                            "p (k o) -> p k o", k=kh
                        ),
                    )
                for k in range(KT):
                    nc.vector.tensor_mul(
                        w16q[:, k, :],
                        w16q[:, k, :],
                        s16[:, oq * OCH : (oq + 1) * OCH],
                    )
                w16q_list.append(w16q)

            def emit_mains(x16, tci):
                for qq in range(NOCH):
                    for tt in range(TCH // 128):
                        t0 = tci * TCH + tt * 128
                        ps = psp.tile([128, OCH], F32)
                        for k in range(KT):
                            nc.tensor.matmul(
                                ps[:],
                                x16[:, k, tt * 128 : tt * 128 + 128],
                                w16q_list[qq][:, k, :],
                                start=(k == 0),
                                stop=False,
                            )
                        nc.tensor.matmul(
                            ps[:],
                            t1sb[:, t0 : t0 + 128],
                            corr16[:, qq * OCH : (qq + 1) * OCH],
                            start=False,
                            stop=True,
                        )
                        ob = obp.tile([128, OCH], F32)
                        if qq % 2 == 0:
                            nc.vector.tensor_copy(ob[:], ps[:])
                        else:
                            nc.scalar.activation(ob[:], ps[:], Copy)
                        nc.scalar.dma_start(out_d[qq, t0 : t0 + 128, :], ob[:])

            prev = None
            for tci in range(NTCH):
                x16 = x16p.tile([128, KT, TCH], F16)
                nc.sync.dma_start(
                    x16[:],
                    xS_d[tci].rearrange("p (k t) -> p k t", k=KT),
                )
                # [t1 | xg] for this chunk
                t1ps = pslp.tile([CW, TCH], F32)
                for k in range(KT):
                    nc.tensor.matmul(
                        t1ps[:],
                        laind[:, k, :],
                        x16[:, k, :],
                        start=(k == 0),
                        stop=(k == KT - 1),
                    )
                nc.vector.tensor_copy(
                    t1sb[0:CW, tci * TCH : (tci + 1) * TCH], t1ps[:]
                )
                if prev is not None:
                    emit_mains(*prev)
                prev = (x16, tci)
            emit_mains(*prev)

    nc.compile()
    return nc


def make_in_maps(x, W_q, scale, zero, lora_A, lora_B, bias):
    x = np.asarray(x, dtype=np.float32)
    W_q = np.asarray(W_q, dtype=np.int32)
    scale = np.asarray(scale, dtype=np.float32)
    zero = np.asarray(zero, dtype=np.float32)
    lora_A = np.asarray(lora_A, dtype=np.float32)
    lora_B = np.asarray(lora_B, dtype=np.float32)
    bias = np.asarray(bias, dtype=np.float32)

    # laind [128, KT*CW]: [2*lora_A | group indicator]
    laP = (SCALING * lora_A[PERM]).astype(np.float16)  # [(k p), R]
    laind = np.zeros((128, KT, CW), dtype=np.float16)
    laind[:, :, 0:R] = laP.reshape(KT, 128, R).transpose(1, 0, 2)
    for pp in range(128):
        laind[pp, :, R + pp % G] = 1.0
    laind = np.ascontiguousarray(laind.reshape(128, KT * CW))

    # xS per batch element (shared by the 2 o-group cores):
    # xS[tci, p, k*TCH+t] = x[b, tci*TCH+t, PERM[k*128+p]]
    xS_b = []
    for b in range(B):
        xT16 = x[b].T[PERM].astype(np.float16)  # [(k p), T]
        xS = (
            xT16.reshape(KT, 128, NTCH, TCH)
            .transpose(2, 1, 0, 3)
            .reshape(NTCH, 128, KT * TCH)
        )
        xS_b.append(np.ascontiguousarray(xS))
    ones16 = np.ones((1, T), dtype=np.float16)

    in_maps = []
    for c in range(NCORES):
        b, og = c // OG, c % OG
        osl = slice(og * O_SH, (og + 1) * O_SH)
        corr = np.empty((CK, O_SH), dtype=np.float16)
        corr[0:R] = lora_B[:, osl]
        corr[R : R + G] = -(zero[osl] * scale[osl]).T
        corr[R + G] = bias[osl]
        # wqS[oq, p, k*OCH+o] = W_q[og*O_SH + oq*OCH + o, PERM[k*128+p]]
        wqT16 = W_q[osl].T[PERM].astype(np.float16)  # [(k p), O_SH]
        wqS = (
            wqT16.reshape(KT, 128, NOCH, OCH)
            .transpose(2, 1, 0, 3)
            .reshape(NOCH, 128, KT * OCH)
        )
        in_maps.append(
            {
                "xS": xS_b[b],
                "wqS": np.ascontiguousarray(wqS),
                "scaleT": np.ascontiguousarray(scale[osl].T.astype(np.float16)),
                "laind": laind,
                "corr": corr,
                "ones": ones16,
            }
        )
    return in_maps


def kernel(x, W_q, scale, zero, lora_A, lora_B, bias):
    global _nc_cache, LAST_RESULTS
    if _nc_cache is None:
        _nc_cache = _build()
    nc = _nc_cache

    in_maps = make_in_maps(x, W_q, scale, zero, lora_A, lora_B, bias)

    res = run_bass_kernel_spmd(
        nc,
        in_maps,
        core_ids=list(range(NCORES)),
        trace=TRACE,
        trace_kwargs=TRACE_KWARGS,
    )
    LAST_RESULTS = res

    out = np.empty((B, S, O), dtype=np.float32)
    for c in range(NCORES):
        b, og = c // OG, c % OG
        o_c = res.results[c]["out"]  # [NOCH, T, OCH]
        for q in range(NOCH):
            o0 = og * O_SH + q * OCH
            out[b, :, o0 : o0 + OCH] = o_c[q]
    return out


# revision 15
# speedup vs baseline: 1.0949x; 1.0519x over previous
"""HQQ-quantized linear + LoRA (nn_HQQLinearLoRA) on 8 trn2 NeuronCores.

  out = x @ ((W_q - zero)*scale)^T + (x @ lora_A @ lora_B) * 2.0 + bias

Sharding: 4 token-groups (batch dim) x 2 out-feature-groups = 8 cores.
Each core computes out[b, :, og*2048:(og+1)*2048] for its (b, og).

Zero-point fold: (W_q - z)*s = W_q*s - z*s, so
  out = x @ (W_q*s)^T - xg @ (z*s)^T + 2*(x @ A) @ B + bias
where xg[t,g] = sum_{i in group g} x[t,i].  Device dequant is then a single
DVE multiply, and xg rides along with the LoRA intermediate: one matmul per
k-tile with lhsT = [2A | group-indicator] produces [t1 | xg] in PSUM, and
one K=81 matmul per output tile applies [lora_B; -(z*s)^T; bias].

Host passes pre-packed f16 views (layout + dtype staging only; W_q values
0..15 are exact in f16).  Rows are permuted so k-tile partition p holds
quant-group g = p % 64 (uniform across k-tiles).  All DMAs are plain HWDGE
transfers (nc.sync / nc.scalar) with per-partition-contiguous descriptors —
no casts, no SWDGE.

All 4 dequantized W quarters stay resident in SBUF (x streamed once);  the
main-matmul stream is software-pipelined one token-chunk behind the
[t1|xg] stage so W loads + dequant hide under the pipeline fill.
"""

import sys

import numpy as np

sys.path.append("/opt/trn_rl_repo")

import concourse.bass as bass  # noqa: E402
import concourse.mybir as mybir  # noqa: E402
import concourse.tile as tile  # noqa: E402
from concourse import bacc  # noqa: E402
from concourse.bass_utils import run_bass_kernel_spmd  # noqa: E402

B, S, I, O, R = 4, 2048, 4096, 4096, 16
GS = 64
G = I // GS  # 64
NCORES = 8
OG = 2
O_SH = O // OG  # 2048
T = S  # 2048 tokens per core
KT = I // 128  # 32 k-tiles
TCH = 256  # token chunk
NTCH = T // TCH  # 8
OCH = 512  # o quarter (dequant-W granule)
NOCH = O_SH // OCH  # 4
SCALING = 2.0
CW = R + G  # 80: [t1 | xg] columns
CK = CW + 1  # 81: correction contraction (incl. ones row for bias)

F32 = mybir.dt.float32
F16 = mybir.dt.float16

TRACE = False
TRACE_KWARGS = {}
LAST_RESULTS = None


def _perm() -> np.ndarray:
    """Row order such that k-tile k, partition p holds input-feature
    i = (p % 64)*64 + 2k + p//64, i.e. quant group g(i) = p % 64."""
    p = np.arange(128)
    out = np.empty(I, dtype=np.int64)
    for k in range(KT):
        out[k * 128 + p] = (p % 64) * 64 + 2 * k + p // 64
    return out


PERM = _perm()

_nc_cache = None


def _build():
    nc = bacc.Bacc(None)
    # xS[tci, p, k*TCH + t] = x[b, tci*TCH + t, PERM[k*128+p]]  (f16)
    xS_d = nc.dram_tensor("xS", [NTCH, 128, KT * TCH], F16, kind="ExternalInput")
    # wqS[oq, p, k*OCH + o] = W_q[og*O_SH + oq*OCH + o, PERM[k*128+p]]  (f16)
    wqS_d = nc.dram_tensor("wqS", [NOCH, 128, KT * OCH], F16, kind="ExternalInput")
    scaleT_d = nc.dram_tensor("scaleT", [G, O_SH], F16, kind="ExternalInput")
    # laind[p, k*CW + 0:R] = 2*lora_A[PERM[k*128+p], :];  [.., R+g] = (p%64==g)
    laind_d = nc.dram_tensor("laind", [128, KT * CW], F16, kind="ExternalInput")
    # corr rows: 0..R-1 = lora_B, R+g = -(zero*scale)^T, R+G = bias
    corr_d = nc.dram_tensor("corr", [CK, O_SH], F16, kind="ExternalInput")
    ones_d = nc.dram_tensor("ones", [1, T], F16, kind="ExternalInput")
    out_d = nc.dram_tensor("out", [NOCH, T, OCH], F32, kind="ExternalOutput")

    Copy = mybir.ActivationFunctionType.Copy

    with tile.TileContext(nc) as tc:
        with (
            tc.tile_pool(name="const", bufs=1) as constp,
            tc.tile_pool(name="w16", bufs=4) as w16p,
            tc.tile_pool(name="x16", bufs=2) as x16p,
            tc.tile_pool(name="ob", bufs=4) as obp,
            tc.tile_pool(name="ps", bufs=6, space="PSUM") as psp,
            tc.tile_pool(name="psl", bufs=2, space="PSUM") as pslp,
        ):
            # All loads share the SP (sync) HWDGE ring: its FIFO order paces
            # HBM so the critical path (s16 -> W q0 -> laind/corr -> x0)
            # lands first and later W quarters trail just ahead of first use.
            # Out stores + PSUM->SBUF copies live on the ACT (scalar) engine;
            # DVE is dedicated to dequant + [t1|xg] casts.
            KH = KT // 4  # W sub-DMA granule (1 MiB) so dequant trails DMA

            def emit_w_dmas(oq, w16q):
                for h in range(4):
                    nc.sync.dma_start(
                        w16q[:, h * KH : (h + 1) * KH, :],
                        wqS_d[oq, :, h * KH * OCH : (h + 1) * KH * OCH].rearrange(
                            "p (k o) -> p k o", k=KH
                        ),
                    )

            def emit_deq(oq, w16q):
                for k in range(KT):
                    nc.vector.tensor_mul(
                        w16q[:, k, :],
                        w16q[:, k, :],
                        s16[:, oq * OCH : (oq + 1) * OCH],
                    )

            s16 = constp.tile([128, O_SH], F16)
            for h in (0, 1):
                nc.sync.dma_start(s16[64 * h : 64 * h + 64, :], scaleT_d[:, :])
            w16q_list = [
                w16p.tile([128, KT, OCH], F16, name="w16q") for i in range(NOCH)
            ]
            emit_w_dmas(0, w16q_list[0])
            emit_deq(0, w16q_list[0])
            laind = constp.tile([128, KT, CW], F16)
            nc.sync.dma_start(laind[:], laind_d.rearrange("p (k c) -> p k c", k=KT))
            corr16 = constp.tile([CK, O_SH], F16)
            nc.sync.dma_start(corr16[:], corr_d[:])
            # [t1 | xg ; ones] lhsT rows; row CW stays 1.0
            t1sb = constp.tile([CK, T], F16)
            nc.sync.dma_start(t1sb[CW : CW + 1, :], ones_d[:])
            x0 = x16p.tile([128, KT, TCH], F16, name="xch")
            nc.sync.dma_start(x0[:], xS_d[0].rearrange("p (k t) -> p k t", k=KT))
            for oq in range(1, NOCH):
                emit_w_dmas(oq, w16q_list[oq])

            def emit_mains(x16, tci):
                for qq in range(NOCH):
                    for tt in range(TCH // 128):
                        t0 = tci * TCH + tt * 128
                        ps = psp.tile([128, OCH], F32)
                        for k in range(KT):
                            nc.tensor.matmul(
                                ps[:],
                                x16[:, k, tt * 128 : tt * 128 + 128],
                                w16q_list[qq][:, k, :],
                                start=(k == 0),
                                stop=False,
                            )
                        nc.tensor.matmul(
                            ps[:],
                            t1sb[:, t0 : t0 + 128],
                            corr16[:, qq * OCH : (qq + 1) * OCH],
                            start=False,
                            stop=True,
                        )
                        ob = obp.tile([128, OCH], F32)
                        nc.scalar.activation(ob[:], ps[:], Copy)
                        nc.scalar.dma_start(out_d[qq, t0 : t0 + 128, :], ob[:])

            x16 = x0
            for tci in range(NTCH):
                # [t1 | xg] for this chunk
                t1ps = pslp.tile([CW, TCH], F32)
                for k in range(KT):
                    nc.tensor.matmul(
                        t1ps[:],
                        laind[:, k, :],
                        x16[:, k, :],
                        start=(k == 0),
                        stop=(k == KT - 1),
                    )
                nc.vector.tensor_copy(
                    t1sb[0:CW, tci * TCH : (tci + 1) * TCH], t1ps[:]
                )
                if tci == 0:
                    # dequant the trailing W quarters; emitted after the
                    # first cast so the strict-FIFO DVE queue can't block it
                    for oq in range(1, NOCH):
                        emit_deq(oq, w16q_list[oq])
                x_next = None
                if tci + 1 < NTCH:
                    x_next = x16p.tile([128, KT, TCH], F16, name="xch")
                    nc.sync.dma_start(
                        x_next[:],
                        xS_d[tci + 1].rearrange("p (k t) -> p k t", k=KT),
                    )
                emit_mains(x16, tci)
                x16 = x_next

    nc.compile()
    return nc


def make_in_maps(x, W_q, scale, zero, lora_A, lora_B, bias):
    x = np.asarray(x, dtype=np.float32)
    W_q = np.asarray(W_q, dtype=np.int32)
    scale = np.asarray(scale, dtype=np.float32)
    zero = np.asarray(zero, dtype=np.float32)
    lora_A = np.asarray(lora_A, dtype=np.float32)
    lora_B = np.asarray(lora_B, dtype=np.float32)
    bias = np.asarray(bias, dtype=np.float32)

    # laind [128, KT*CW]: [2*lora_A | group indicator]
    laP = (SCALING * lora_A[PERM]).astype(np.float16)  # [(k p), R]
    laind = np.zeros((128, KT, CW), dtype=np.float16)
    laind[:, :, 0:R] = laP.reshape(KT, 128, R).transpose(1, 0, 2)
    for pp in range(128):
        laind[pp, :, R + pp % G] = 1.0
    laind = np.ascontiguousarray(laind.reshape(128, KT * CW))

    # xS per batch element (shared by the 2 o-group cores):
    # xS[tci, p, k*TCH+t] = x[b, tci*TCH+t, PERM[k*128+p]]
    xS_b = []
    for b in range(B):
        xT16 = x[b].T[PERM].astype(np.float16)  # [(k p), T]
        xS = (
            xT16.reshape(KT, 128, NTCH, TCH)
            .transpose(2, 1, 0, 3)
            .reshape(NTCH, 128, KT * TCH)
        )
        xS_b.append(np.ascontiguousarray(xS))
    ones16 = np.ones((1, T), dtype=np.float16)

    in_maps = []
    for c in range(NCORES):
        b, og = c // OG, c % OG
        osl = slice(og * O_SH, (og + 1) * O_SH)
        corr = np.empty((CK, O_SH), dtype=np.float16)
        corr[0:R] = lora_B[:, osl]
        corr[R : R + G] = -(zero[osl] * scale[osl]).T
        corr[R + G] = bias[osl]
        # wqS[oq, p, k*OCH+o] = W_q[og*O_SH + oq*OCH + o, PERM[k*128+p]]
        wqT16 = W_q[osl].T[PERM].astype(np.float16)  # [(k p), O_SH]
        wqS = (
            wqT16.reshape(KT, 128, NOCH, OCH)
            .transpose(2, 1, 0, 3)
            .reshape(NOCH, 128, KT * OCH)
        )
        in_maps.append(
            {
                "xS": xS_b[b],
                "wqS": np.ascontiguousarray(wqS),
                "scaleT": np.ascontiguousarray(scale[osl].T.astype(np.float16)),
                "laind": laind,
                "corr": corr,
                "ones": ones16,
            }
        )
    return in_maps


def kernel(x, W_q, scale, zero, lora_A, lora_B, bias):
    global _nc_cache, LAST_RESULTS
    if _nc_cache is None:
        _nc_cache = _build()
    nc = _nc_cache

    in_maps = make_in_maps(x, W_q, scale, zero, lora_A, lora_B, bias)

    res = run_bass_kernel_spmd(
        nc,
        in_maps,
        core_ids=list(range(NCORES)),
        trace=TRACE,
        trace_kwargs=TRACE_KWARGS,
    )
    LAST_RESULTS = res

    out = np.empty((B, S, O), dtype=np.float32)
    for c in range(NCORES):
        b, og = c // OG, c % OG
        o_c = res.results[c]["out"]  # [NOCH, T, OCH]
        for q in range(NOCH):
            o0 = og * O_SH + q * OCH
            out[b, :, o0 : o0 + OCH] = o_c[q]
    return out
